# revision 2
# baseline (speedup 1.0000x reference)
"""Trainium2 Bass kernel for the DecoderSVM SNN decoder.

reference computation:
    curr[t,b,o] = einsum('bit,oi->tbo', inputs, W) + b         (I=182 -> O=2)
    syn_t = clip(alpha,0,1)*syn_{t-1} + curr_t                 (scan over T)
    mem_t = clip(beta,0,1)*mem_{t-1} + syn_t
    out = mem_rec transposed to [B, T, O]

Strategy (8 NeuronCores, batch-sharded 32 per core):
  - Block-diagonal GEMM: K=128 partitions = 32 batches x 4 input rows; the
    stationary lhsT [128, 64] holds W values block-diagonally so one matmul
    emits PSUM [64=(b,o), N] -- exactly the (batch,o)-per-partition layout
    the scan needs.  46 row-chunks accumulate the full I=182 contraction.
  - Bias enters PSUM via a rank-1 matmul: ones[1,N] x bias_row[1,64].
  - The double recurrence = two chained first-order linear scans done with
    VectorE's native tensor_tensor_scan (state = a*state + c) straight out
    of PSUM.
  - Output [64, 2000] DMAs contiguously; host reassembles [B, T, O].
"""

import numpy as np

B, I, T, O = 256, 182, 2000, 2
NCORES = 8
NB = B // NCORES          # 32 batches per core
ROWS = 4                  # input rows folded into K per full chunk
NFULL = I // ROWS         # 45 full chunks (180 rows)
LAST_ROWS = I - NFULL * ROWS   # 2 rows in the tail chunk
M = 2 * NB                # 64 = output partitions (b_local, o)
TSPLIT = [512, 512, 512, 464]  # PSUM-bank-aligned time tiles

# "f32"  : exact fp32 matmuls (4 cyc/row on PE)
# "f32r" : fp32 data, float32r matmul mode (1 cyc/row at N>=256)
# "bf16" : host-cast inputs to bf16 (halves DMA traffic)
MODE = "f32"
TRACE = False

_cache = {}


def _dt():
    import concourse.mybir as mybir

    return {
        "f32": mybir.dt.float32,
        "f32r": mybir.dt.float32r,
        "bf16": mybir.dt.bfloat16,
    }[MODE]


def _np_dt():
    if MODE == "bf16":
        import ml_dtypes

        return ml_dtypes.bfloat16
    return np.float32


def _build_nc():
    import concourse.bacc as bacc
    import concourse.bass as bass
    import concourse.mybir as mybir
    from concourse.tile import TileContext

    f32 = mybir.dt.float32
    mdt = _dt()

    nc = bacc.Bacc("TRN2", target_bir_lowering=False, debug=False)

    x = nc.dram_tensor("x", [NB, I, T], mdt, kind="ExternalInput")
    lhsT_full = nc.dram_tensor("lhsT_full", [128, NFULL * M], mdt, kind="ExternalInput")
    lhsT_last = nc.dram_tensor("lhsT_last", [2 * NB, M], mdt, kind="ExternalInput")
    bias_row = nc.dram_tensor("bias_row", [1, M], mdt, kind="ExternalInput")
    alpha_bc = nc.dram_tensor("alpha_bc", [M, 512], f32, kind="ExternalInput")
    beta_bc = nc.dram_tensor("beta_bc", [M, 512], f32, kind="ExternalInput")
    y = nc.dram_tensor("y", [M, T], f32, kind="ExternalOutput")

    with TileContext(nc) as tc:
        with (
            tc.tile_pool(name="consts", bufs=1) as cpool,
            tc.tile_pool(name="xs", bufs=4) as xpool,
            tc.tile_pool(name="xl", bufs=1) as xlpool,
            tc.tile_pool(name="mems", bufs=1) as mpool,
            tc.tile_pool(name="psum", bufs=1, space=bass.MemorySpace.PSUM) as ppool,
        ):
            lw = cpool.tile([128, NFULL, M], mdt)
            nc.sync.dma_start(out=lw[:], in_=lhsT_full[:])
            lwl = cpool.tile([2 * NB, M], mdt)
            nc.sync.dma_start(out=lwl[:], in_=lhsT_last[:])
            br = cpool.tile([1, M], mdt)
            nc.sync.dma_start(out=br[:], in_=bias_row[:])
            ab = cpool.tile([M, 512], f32)
            nc.sync.dma_start(out=ab[:], in_=alpha_bc[:])
            bb = cpool.tile([M, 512], f32)
            nc.sync.dma_start(out=bb[:], in_=beta_bc[:])
            ones = cpool.tile([1, T], mdt)
            nc.vector.memset(ones[:], 1.0)

            pt = ppool.tile([M, 2048], f32)

            for c in range(NFULL):
                xt = xpool.tile([128, T], mdt, tag="xt")
                nc.sync.dma_start(out=xt[:], in_=x[:, ROWS * c : ROWS * (c + 1), :])
                off = 0
                for w in TSPLIT:
                    nc.tensor.matmul(
                        pt[:, off : off + w],
                        lw[:, c, :],
                        xt[:, off : off + w],
                        start=(c == 0),
                        stop=False,
                    )
                    off += w
            # tail chunk: rows 180..182, K = 32 batches * 2 rows = 64
            xt2 = xlpool.tile([2 * NB, T], mdt)
            nc.sync.dma_start(out=xt2[:], in_=x[:, NFULL * ROWS :, :])
            off = 0
            for w in TSPLIT:
                nc.tensor.matmul(
                    pt[:, off : off + w],
                    lwl[:],
                    xt2[:, off : off + w],
                    start=False,
                    stop=False,
                )
                off += w
            # bias: ones[1, N] (x) bias_row[1, 64]
            off = 0
            for w in TSPLIT:
                nc.tensor.matmul(
                    pt[:, off : off + w],
                    br[:],
                    ones[:, off : off + w],
                    start=False,
                    stop=True,
                )
                off += w

            syn = mpool.tile([M, T], f32)
            mem = mpool.tile([M, T], f32)
            off = 0
            for ti, w in enumerate(TSPLIT):
                nc.vector.tensor_tensor_scan(
                    syn[:, off : off + w],
                    ab[:, :w],
                    pt[:, off : off + w],
                    initial=(0.0 if ti == 0 else syn[:, off - 1 : off]),
                    op0=mybir.AluOpType.mult,
                    op1=mybir.AluOpType.add,
                )
                off += w
            off = 0
            for ti, w in enumerate(TSPLIT):
                nc.vector.tensor_tensor_scan(
                    mem[:, off : off + w],
                    bb[:, :w],
                    syn[:, off : off + w],
                    initial=(0.0 if ti == 0 else mem[:, off - 1 : off]),
                    op0=mybir.AluOpType.mult,
                    op1=mybir.AluOpType.add,
                )
                off += w

            nc.sync.dma_start(out=y[:], in_=mem[:])

    nc.compile()
    return nc


def _host_tensors(W, b, alpha, beta):
    """Build the block-diagonal stationary weights + scan constant tensors."""
    npdt = _np_dt()
    W = np.asarray(W, np.float32)
    bvec = np.asarray(b, np.float32)
    a_cl = np.clip(np.asarray(alpha, np.float32), 0.0, 1.0)
    bt_cl = np.clip(np.asarray(beta, np.float32), 0.0, 1.0)

    bidx = np.arange(NB)
    lhsT = np.zeros((128, NFULL, M), np.float32)
    for c in range(NFULL):
        for i in range(ROWS):
            for o in range(O):
                lhsT[ROWS * bidx + i, c, 2 * bidx + o] = W[o, ROWS * c + i]
    lhsT_full = lhsT.reshape(128, NFULL * M).astype(npdt)

    lhsT_last = np.zeros((2 * NB, M), np.float32)
    for i in range(LAST_ROWS):
        for o in range(O):
            lhsT_last[LAST_ROWS * bidx + i, 2 * bidx + o] = W[o, NFULL * ROWS + i]
    lhsT_last = lhsT_last.astype(npdt)

    bias_row = np.tile(bvec, NB)[None, :].astype(npdt)
    alpha_bc = np.ascontiguousarray(
        np.broadcast_to(np.tile(a_cl, NB)[:, None], (M, 512))
    ).astype(np.float32)
    beta_bc = np.ascontiguousarray(
        np.broadcast_to(np.tile(bt_cl, NB)[:, None], (M, 512))
    ).astype(np.float32)
    return lhsT_full, lhsT_last, bias_row, alpha_bc, beta_bc


def kernel(inputs, W, b, alpha, beta):
    from concourse.bass_utils import run_bass_kernel_spmd

    key = MODE
    if key not in _cache:
        _cache[key] = _build_nc()
    nc = _cache[key]

    npdt = _np_dt()
    lhsT_full, lhsT_last, bias_row, alpha_bc, beta_bc = _host_tensors(
        W, b, alpha, beta
    )
    x_full = np.asarray(inputs, np.float32)
    if MODE == "bf16":
        x_full = x_full.astype(npdt)

    in_maps = []
    for c in range(NCORES):
        in_maps.append(
            {
                "x": np.ascontiguousarray(x_full[c * NB : (c + 1) * NB]),
                "lhsT_full": lhsT_full,
                "lhsT_last": lhsT_last,
                "bias_row": bias_row,
                "alpha_bc": alpha_bc,
                "beta_bc": beta_bc,
            }
        )

    res = run_bass_kernel_spmd(nc, in_maps, core_ids=list(range(NCORES)), trace=TRACE)
    kernel.last_exec_time_ns = res.exec_time_ns
    out = np.empty((B, O, T), np.float32)
    for c in range(NCORES):
        out[c * NB : (c + 1) * NB] = res.results[c]["y"].reshape(NB, O, T)
    return np.ascontiguousarray(out.transpose(0, 2, 1))


kernel.last_exec_time_ns = None


# revision 4
# speedup vs baseline: 1.0832x; 1.0832x over previous
"""Trainium2 Bass kernel for the DecoderSVM SNN decoder.

reference computation:
    curr[t,b,o] = einsum('bit,oi->tbo', inputs, W) + b         (I=182 -> O=2)
    syn_t = clip(alpha,0,1)*syn_{t-1} + curr_t                 (scan over T)
    mem_t = clip(beta,0,1)*mem_{t-1} + syn_t
    out = mem_rec transposed to [B, T, O]

Strategy (8 NeuronCores, batch-sharded 32 per core):
  - Block-diagonal GEMM: K=128 partitions = 32 batches x 4 input rows; the
    stationary lhsT [128, 64] holds W values block-diagonally so one matmul
    emits PSUM [64=(b,o), N] -- exactly the (batch,o)-per-partition layout
    the scan needs.  46 row-chunks accumulate the full I=182 contraction.
  - Bias enters PSUM via a rank-1 matmul: ones[1,N] x bias_row[1,64].
  - The double recurrence = two chained first-order linear scans done with
    VectorE's native tensor_tensor_scan (state = a*state + c) straight out
    of PSUM.
  - Output [64, 2000] DMAs contiguously; host reassembles [B, T, O].
"""

import numpy as np

B, I, T, O = 256, 182, 2000, 2
NCORES = 8
NB = B // NCORES          # 32 batches per core
ROWS = 4                  # input rows folded into K per full chunk
NFULL = I // ROWS         # 45 full chunks (180 rows)
LAST_ROWS = I - NFULL * ROWS   # 2 rows in the tail chunk
M = 2 * NB                # 64 = output partitions (b_local, o)
TSPLIT = [512, 512, 512, 464]  # PSUM-bank-aligned time tiles

# "f32"  : exact fp32 matmuls (4 cyc/row on PE)
# "f32r" : fp32 data, float32r matmul mode (1 cyc/row at N>=256)
# "bf16" : host-cast inputs to bf16 (halves DMA traffic)
MODE = "f32"
TRACE = False

_cache = {}


def _dt():
    import concourse.mybir as mybir

    return {
        "f32": mybir.dt.float32,
        "f32r": mybir.dt.float32r,
        "bf16": mybir.dt.bfloat16,
    }[MODE]


def _np_dt():
    if MODE == "bf16":
        import ml_dtypes

        return ml_dtypes.bfloat16
    return np.float32


def _build_nc():
    import concourse.bacc as bacc
    import concourse.bass as bass
    import concourse.mybir as mybir
    from concourse.tile import TileContext

    f32 = mybir.dt.float32
    mdt = _dt()
    # float32r memset is not encodable; the bias rank-1 matmul stays plain f32
    bdt = f32 if MODE == "f32r" else mdt

    nc = bacc.Bacc("TRN2", target_bir_lowering=False, debug=False)

    x = nc.dram_tensor("x", [NB, I, T], mdt, kind="ExternalInput")
    lhsT_full = nc.dram_tensor("lhsT_full", [128, NFULL * M], mdt, kind="ExternalInput")
    lhsT_last = nc.dram_tensor("lhsT_last", [2 * NB, M], mdt, kind="ExternalInput")
    bias_row = nc.dram_tensor("bias_row", [1, M], bdt, kind="ExternalInput")
    alpha_bc = nc.dram_tensor("alpha_bc", [M, 512], f32, kind="ExternalInput")
    beta_bc = nc.dram_tensor("beta_bc", [M, 512], f32, kind="ExternalInput")
    y = nc.dram_tensor("y", [M, T], f32, kind="ExternalOutput")

    with TileContext(nc) as tc:
        with (
            tc.tile_pool(name="consts", bufs=1) as cpool,
            tc.tile_pool(name="xs", bufs=4) as xpool,
            tc.tile_pool(name="xl", bufs=1) as xlpool,
            tc.tile_pool(name="mems", bufs=1) as mpool,
            tc.tile_pool(name="psum", bufs=1, space=bass.MemorySpace.PSUM) as ppool,
        ):
            lw = cpool.tile([128, NFULL, M], mdt)
            nc.sync.dma_start(out=lw[:], in_=lhsT_full[:])
            lwl = cpool.tile([2 * NB, M], mdt)
            nc.sync.dma_start(out=lwl[:], in_=lhsT_last[:])
            br = cpool.tile([1, M], bdt)
            nc.sync.dma_start(out=br[:], in_=bias_row[:])
            ab = cpool.tile([M, 512], f32)
            nc.sync.dma_start(out=ab[:], in_=alpha_bc[:])
            bb = cpool.tile([M, 512], f32)
            nc.sync.dma_start(out=bb[:], in_=beta_bc[:])
            ones = cpool.tile([1, T], bdt)
            nc.vector.memset(ones[:], 1.0)

            pt = ppool.tile([M, 2048], f32)

            for c in range(NFULL):
                xt = xpool.tile([128, T], mdt, tag="xt")
                nc.sync.dma_start(out=xt[:], in_=x[:, ROWS * c : ROWS * (c + 1), :])
                off = 0
                for w in TSPLIT:
                    nc.tensor.matmul(
                        pt[:, off : off + w],
                        lw[:, c, :],
                        xt[:, off : off + w],
                        start=(c == 0),
                        stop=False,
                    )
                    off += w
            # tail chunk: rows 180..182, K = 32 batches * 2 rows = 64
            xt2 = xlpool.tile([2 * NB, T], mdt)
            nc.sync.dma_start(out=xt2[:], in_=x[:, NFULL * ROWS :, :])
            off = 0
            for w in TSPLIT:
                nc.tensor.matmul(
                    pt[:, off : off + w],
                    lwl[:],
                    xt2[:, off : off + w],
                    start=False,
                    stop=False,
                )
                off += w
            # bias: ones[1, N] (x) bias_row[1, 64]
            off = 0
            for w in TSPLIT:
                nc.tensor.matmul(
                    pt[:, off : off + w],
                    br[:],
                    ones[:, off : off + w],
                    start=False,
                    stop=True,
                )
                off += w

            syn = mpool.tile([M, T], f32)
            mem = mpool.tile([M, T], f32)
            off = 0
            for ti, w in enumerate(TSPLIT):
                nc.vector.tensor_tensor_scan(
                    syn[:, off : off + w],
                    ab[:, :w],
                    pt[:, off : off + w],
                    initial=(0.0 if ti == 0 else syn[:, off - 1 : off]),
                    op0=mybir.AluOpType.mult,
                    op1=mybir.AluOpType.add,
                )
                off += w
            off = 0
            for ti, w in enumerate(TSPLIT):
                nc.vector.tensor_tensor_scan(
                    mem[:, off : off + w],
                    bb[:, :w],
                    syn[:, off : off + w],
                    initial=(0.0 if ti == 0 else mem[:, off - 1 : off]),
                    op0=mybir.AluOpType.mult,
                    op1=mybir.AluOpType.add,
                )
                off += w

            nc.sync.dma_start(out=y[:], in_=mem[:])

    nc.compile()
    return nc


def _host_tensors(W, b, alpha, beta):
    """Build the block-diagonal stationary weights + scan constant tensors."""
    npdt = _np_dt()
    W = np.asarray(W, np.float32)
    bvec = np.asarray(b, np.float32)
    a_cl = np.clip(np.asarray(alpha, np.float32), 0.0, 1.0)
    bt_cl = np.clip(np.asarray(beta, np.float32), 0.0, 1.0)

    bidx = np.arange(NB)
    lhsT = np.zeros((128, NFULL, M), np.float32)
    for c in range(NFULL):
        for i in range(ROWS):
            for o in range(O):
                lhsT[ROWS * bidx + i, c, 2 * bidx + o] = W[o, ROWS * c + i]
    lhsT_full = lhsT.reshape(128, NFULL * M).astype(npdt)

    lhsT_last = np.zeros((2 * NB, M), np.float32)
    for i in range(LAST_ROWS):
        for o in range(O):
            lhsT_last[LAST_ROWS * bidx + i, 2 * bidx + o] = W[o, NFULL * ROWS + i]
    lhsT_last = lhsT_last.astype(npdt)

    bias_row = np.tile(bvec, NB)[None, :].astype(npdt)
    alpha_bc = np.ascontiguousarray(
        np.broadcast_to(np.tile(a_cl, NB)[:, None], (M, 512))
    ).astype(np.float32)
    beta_bc = np.ascontiguousarray(
        np.broadcast_to(np.tile(bt_cl, NB)[:, None], (M, 512))
    ).astype(np.float32)
    return lhsT_full, lhsT_last, bias_row, alpha_bc, beta_bc


def kernel(inputs, W, b, alpha, beta):
    from concourse.bass_utils import run_bass_kernel_spmd

    key = MODE
    if key not in _cache:
        _cache[key] = _build_nc()
    nc = _cache[key]

    npdt = _np_dt()
    lhsT_full, lhsT_last, bias_row, alpha_bc, beta_bc = _host_tensors(
        W, b, alpha, beta
    )
    x_full = np.asarray(inputs, np.float32)
    if MODE == "bf16":
        x_full = x_full.astype(npdt)

    in_maps = []
    for c in range(NCORES):
        in_maps.append(
            {
                "x": np.ascontiguousarray(x_full[c * NB : (c + 1) * NB]),
                "lhsT_full": lhsT_full,
                "lhsT_last": lhsT_last,
                "bias_row": bias_row,
                "alpha_bc": alpha_bc,
                "beta_bc": beta_bc,
            }
        )

    res = run_bass_kernel_spmd(nc, in_maps, core_ids=list(range(NCORES)), trace=TRACE)
    kernel.last_exec_time_ns = res.exec_time_ns
    out = np.empty((B, O, T), np.float32)
    for c in range(NCORES):
        out[c * NB : (c + 1) * NB] = res.results[c]["y"].reshape(NB, O, T)
    return np.ascontiguousarray(out.transpose(0, 2, 1))


kernel.last_exec_time_ns = None


# revision 5
# speedup vs baseline: 1.8484x; 1.7065x over previous
"""Trainium2 Bass kernel for the DecoderSVM SNN decoder.

reference computation:
    curr[t,b,o] = einsum('bit,oi->tbo', inputs, W) + b         (I=182 -> O=2)
    syn_t = clip(alpha,0,1)*syn_{t-1} + curr_t                 (scan over T)
    mem_t = clip(beta,0,1)*mem_{t-1} + syn_t
    out = mem_rec transposed to [B, T, O]

Strategy (8 NeuronCores, batch-sharded 32 per core):
  - Block-diagonal GEMM: K=128 partitions = 32 batches x 4 input rows; the
    stationary lhsT [128, 64] holds W values block-diagonally so one matmul
    emits PSUM [64=(b,o), N] -- exactly the (batch,o)-per-partition layout
    the scan needs.  46 row-chunks accumulate the full I=182 contraction.
  - Bias enters PSUM via a rank-1 matmul: ones[1,N] x bias_row[1,64].
  - The double recurrence = two chained first-order linear scans done with
    VectorE's native tensor_tensor_scan (state = a*state + c) straight out
    of PSUM.
  - Output [64, 2000] DMAs contiguously; host reassembles [B, T, O].
"""

import numpy as np

B, I, T, O = 256, 182, 2000, 2
NCORES = 8
NB = B // NCORES          # 32 batches per core
ROWS = 4                  # input rows folded into K per full chunk
NFULL = I // ROWS         # 45 full chunks (180 rows)
LAST_ROWS = I - NFULL * ROWS   # 2 rows in the tail chunk
M = 2 * NB                # 64 = output partitions (b_local, o)
TSPLIT = [512, 512, 512, 464]  # PSUM-bank-aligned time tiles

# "f32"  : exact fp32 matmuls (4 cyc/row on PE)
# "f32r" : fp32 data, float32r matmul mode (1 cyc/row at N>=256)
# "bf16" : host-cast inputs to bf16 (halves DMA traffic)
MODE = "f32"
TRACE = False

_cache = {}


def _dt():
    import concourse.mybir as mybir

    return {
        "f32": mybir.dt.float32,
        "f32r": mybir.dt.float32r,
        "bf16": mybir.dt.bfloat16,
    }[MODE]


def _np_dt():
    if MODE == "bf16":
        import ml_dtypes

        return ml_dtypes.bfloat16
    return np.float32


def _build_nc():
    import concourse.bacc as bacc
    import concourse.bass as bass
    import concourse.mybir as mybir
    from concourse.tile import TileContext

    f32 = mybir.dt.float32
    mdt = _dt()
    # float32r memset is not encodable; the bias rank-1 matmul stays plain f32
    bdt = f32 if MODE == "f32r" else mdt

    nc = bacc.Bacc("TRN2", target_bir_lowering=False, debug=False)

    x = nc.dram_tensor("x", [NB, I, T], mdt, kind="ExternalInput")
    lhsT_full = nc.dram_tensor("lhsT_full", [128, NFULL * M], mdt, kind="ExternalInput")
    lhsT_last = nc.dram_tensor("lhsT_last", [2 * NB, M], mdt, kind="ExternalInput")
    bias_row = nc.dram_tensor("bias_row", [1, M], bdt, kind="ExternalInput")
    alpha_bc = nc.dram_tensor("alpha_bc", [M, 512], f32, kind="ExternalInput")
    beta_bc = nc.dram_tensor("beta_bc", [M, 512], f32, kind="ExternalInput")
    y = nc.dram_tensor("y", [M, T], f32, kind="ExternalOutput")

    with TileContext(nc) as tc:
        with (
            tc.tile_pool(name="consts", bufs=1) as cpool,
            tc.tile_pool(name="xs", bufs=4) as xpool,
            tc.tile_pool(name="xl", bufs=1) as xlpool,
            tc.tile_pool(name="mems", bufs=1) as mpool,
            tc.tile_pool(name="psum", bufs=1, space=bass.MemorySpace.PSUM) as ppool,
        ):
            lw = cpool.tile([128, NFULL, M], mdt)
            nc.sync.dma_start(out=lw[:], in_=lhsT_full[:])
            lwl = cpool.tile([2 * NB, M], mdt)
            nc.sync.dma_start(out=lwl[:], in_=lhsT_last[:])
            br = cpool.tile([1, M], bdt)
            nc.sync.dma_start(out=br[:], in_=bias_row[:])
            ab = cpool.tile([M, 512], f32)
            nc.sync.dma_start(out=ab[:], in_=alpha_bc[:])
            bb = cpool.tile([M, 512], f32)
            nc.sync.dma_start(out=bb[:], in_=beta_bc[:])
            ones = cpool.tile([1, T], bdt)
            nc.vector.memset(ones[:], 1.0)

            pt = ppool.tile([M, 2048], f32)

            for c in range(NFULL):
                xt = xpool.tile([128, T], mdt, tag="xt")
                nc.sync.dma_start(out=xt[:], in_=x[:, ROWS * c : ROWS * (c + 1), :])
                off = 0
                for w in TSPLIT:
                    nc.tensor.matmul(
                        pt[:, off : off + w],
                        lw[:, c, :],
                        xt[:, off : off + w],
                        start=(c == 0),
                        stop=False,
                    )
                    off += w
            # tail chunk: rows 180..182, K = 32 batches * 2 rows = 64
            xt2 = xlpool.tile([2 * NB, T], mdt)
            nc.sync.dma_start(out=xt2[:], in_=x[:, NFULL * ROWS :, :])
            off = 0
            for w in TSPLIT:
                nc.tensor.matmul(
                    pt[:, off : off + w],
                    lwl[:],
                    xt2[:, off : off + w],
                    start=False,
                    stop=False,
                )
                off += w
            # bias: ones[1, N] (x) bias_row[1, 64]
            off = 0
            for w in TSPLIT:
                nc.tensor.matmul(
                    pt[:, off : off + w],
                    br[:],
                    ones[:, off : off + w],
                    start=False,
                    stop=True,
                )
                off += w

            syn = mpool.tile([M, T], f32)
            mem = mpool.tile([M, T], f32)
            off = 0
            for ti, w in enumerate(TSPLIT):
                nc.vector.tensor_tensor_scan(
                    syn[:, off : off + w],
                    ab[:, :w],
                    pt[:, off : off + w],
                    initial=(0.0 if ti == 0 else syn[:, off - 1 : off]),
                    op0=mybir.AluOpType.mult,
                    op1=mybir.AluOpType.add,
                )
                off += w
            off = 0
            for ti, w in enumerate(TSPLIT):
                nc.vector.tensor_tensor_scan(
                    mem[:, off : off + w],
                    bb[:, :w],
                    syn[:, off : off + w],
                    initial=(0.0 if ti == 0 else mem[:, off - 1 : off]),
                    op0=mybir.AluOpType.mult,
                    op1=mybir.AluOpType.add,
                )
                off += w

            nc.sync.dma_start(out=y[:], in_=mem[:])

    nc.compile()
    return nc


def _host_tensors(W, b, alpha, beta):
    """Build the block-diagonal stationary weights + scan constant tensors."""
    npdt = _np_dt()
    W = np.asarray(W, np.float32)
    bvec = np.asarray(b, np.float32)
    a_cl = np.clip(np.asarray(alpha, np.float32), 0.0, 1.0)
    bt_cl = np.clip(np.asarray(beta, np.float32), 0.0, 1.0)

    bidx = np.arange(NB)
    lhsT = np.zeros((128, NFULL, M), np.float32)
    for c in range(NFULL):
        for i in range(ROWS):
            for o in range(O):
                lhsT[ROWS * bidx + i, c, 2 * bidx + o] = W[o, ROWS * c + i]
    lhsT_full = lhsT.reshape(128, NFULL * M).astype(npdt)

    lhsT_last = np.zeros((2 * NB, M), np.float32)
    for i in range(LAST_ROWS):
        for o in range(O):
            lhsT_last[LAST_ROWS * bidx + i, 2 * bidx + o] = W[o, NFULL * ROWS + i]
    lhsT_last = lhsT_last.astype(npdt)

    bias_row = np.tile(bvec, NB)[None, :].astype(npdt)
    alpha_bc = np.ascontiguousarray(
        np.broadcast_to(np.tile(a_cl, NB)[:, None], (M, 512))
    ).astype(np.float32)
    beta_bc = np.ascontiguousarray(
        np.broadcast_to(np.tile(bt_cl, NB)[:, None], (M, 512))
    ).astype(np.float32)
    return lhsT_full, lhsT_last, bias_row, alpha_bc, beta_bc


def kernel(inputs, W, b, alpha, beta):
    from concourse.bass_utils import run_bass_kernel_spmd

    key = MODE
    if key not in _cache:
        _cache[key] = _build_nc()
    nc = _cache[key]

    npdt = _np_dt()
    lhsT_full, lhsT_last, bias_row, alpha_bc, beta_bc = _host_tensors(
        W, b, alpha, beta
    )
    x_full = np.asarray(inputs, np.float32)
    if MODE == "bf16":
        x_full = x_full.astype(npdt)

    in_maps = []
    for c in range(NCORES):
        in_maps.append(
            {
                "x": np.ascontiguousarray(x_full[c * NB : (c + 1) * NB]),
                "lhsT_full": lhsT_full,
                "lhsT_last": lhsT_last,
                "bias_row": bias_row,
                "alpha_bc": alpha_bc,
                "beta_bc": beta_bc,
            }
        )

    res = run_bass_kernel_spmd(nc, in_maps, core_ids=list(range(NCORES)), trace=TRACE)
    kernel.last_exec_time_ns = res.exec_time_ns
    kernel.last_result = res
    out = np.empty((B, O, T), np.float32)
    for c in range(NCORES):
        out[c * NB : (c + 1) * NB] = res.results[c]["y"].reshape(NB, O, T)
    return np.ascontiguousarray(out.transpose(0, 2, 1))


kernel.last_exec_time_ns = None
